# revision 5
# baseline (speedup 1.0000x reference)
"""Clustered Linformer Attention — Trainium2 Bass kernel, 8 NeuronCores.

Strategy: data-parallel over batch (2 batches/core, no collectives).
Math restructuring (verified vs reference to ~7e-7 in f32):
  - mask is all-ones => cluster c holds positions [32c, 32c+32); the per-head
    gather+einsum projections become  k_proj = AE[h]^T @ k_h  with a host-built
    sparse table AE[h] in [S, P] (score scale folded in), same for v with AF.
  - the 3-kernel conv fusion over scores collapses to 5 "tap" matrices M_t in
    [P, P] (t in -2..2):  scores_conv[s] = sum_t  (q[s+t] @ (k_proj^T @ M_t)).
    Taps are applied as 5 PSUM-accumulated matmuls with a column-shifted
    (zero-padded) q^T operand.
  - adjacent heads are packed block-diagonally so every matmul contracts over
    the full 128 partitions.
  - softmax has no max-subtraction (|scores| <~ 1.6, exp is safe in f32);
    Z = sum_c exp is computed by an all-ones block-diag matmul that also
    broadcasts Z to all 128 partitions, so normalization is one DVE op.
"""
import sys
import numpy as np
import ml_dtypes

sys.path.insert(0, '/opt/trn_rl_repo')

B, S, D = 16, 2048, 512
H, P, C = 8, 64, 32
DEPTH = D // H           # 64
NCORES = 8
BLOC = B // NCORES       # 2 batches per core
NPAIR = H // 2           # 4 head pairs
SCH = 4                  # s-chunks of 512
SCW = S // SCH           # 512
NJ = S // 128            # 16 s-tiles of 128
NDC = D // 128           # 4 contraction chunks

_CACHE = {}


def _build_nc():
    import concourse.tile as tile
    from concourse import mybir, bacc

    f32 = mybir.dt.float32
    f32r = mybir.dt.float32r
    bf16 = mybir.dt.bfloat16

    nc = bacc.Bacc()
    xT = nc.declare_dram_parameter("xT", [BLOC, D, S], f32r, isOutput=False)
    wq = nc.declare_dram_parameter("wq", [D, D], f32r, isOutput=False)
    wk = nc.declare_dram_parameter("wk", [D, D], f32r, isOutput=False)
    wv = nc.declare_dram_parameter("wv", [D, D], f32r, isOutput=False)
    dw = nc.declare_dram_parameter("dw", [D, D], bf16, isOutput=False)
    db = nc.declare_dram_parameter("db", [D], f32, isOutput=False)
    ae = nc.declare_dram_parameter("ae", [NPAIR, S, 128], bf16, isOutput=False)
    af = nc.declare_dram_parameter("af", [NPAIR, S, 128], bf16, isOutput=False)
    bdm = nc.declare_dram_parameter("bdm", [5, 128, 128], bf16, isOutput=False)
    onesbd = nc.declare_dram_parameter("onesbd", [128, 128], bf16, isOutput=False)
    out = nc.declare_dram_parameter("out", [BLOC, S, D], f32, isOutput=True)

    import concourse.bass as bass

    with tile.TileContext(nc) as tc:
        with tc.tile_pool(name="const", bufs=1) as cpool, \
             tc.tile_pool(name="big", bufs=1) as bigp, \
             tc.tile_pool(name="ah", bufs=2) as ahp, \
             tc.tile_pool(name="sm", bufs=4) as smp, \
             tc.tile_pool(name="bd", bufs=2) as bdp, \
             tc.tile_pool(name="ob", bufs=3) as obp, \
             tc.tile_pool(name="psB", bufs=5, space="PSUM") as psB, \
             tc.tile_pool(name="psS", bufs=2, space="PSUM") as psS:

            # ---- constants in SBUF ----
            wq_sb = cpool.tile([128, NDC, D], f32r)
            wk_sb = cpool.tile([128, NDC, D], f32r)
            wv_sb = cpool.tile([128, NDC, D], f32r)
            dw_sb = cpool.tile([128, NDC, D], bf16)
            for t_sb, t_dr in ((wq_sb, wq), (wk_sb, wk), (wv_sb, wv), (dw_sb, dw)):
                nc.sync.dma_start(out=t_sb, in_=t_dr[:].rearrange("(o p) m -> p o m", p=128))
            bdm_sb = cpool.tile([128, 5, 128], bf16)
            nc.sync.dma_start(out=bdm_sb, in_=bdm[:].rearrange("t p m -> p t m"))
            ones_sb = cpool.tile([128, 128], bf16)
            nc.sync.dma_start(out=ones_sb, in_=onesbd[:])
            bias_sb = cpool.tile([128, D], f32)
            db_ap = db[:]
            nc.gpsimd.dma_start(
                out=bias_sb,
                in_=bass.AP(tensor=db_ap.tensor, offset=db_ap.offset,
                            ap=[[0, 128]] + list(db_ap.ap)))

            for b in range(BLOC):
                # ---- load pre-transposed x ----
                xt_sb = bigp.tile([128, NDC, S], f32r, tag="xT")
                nc.sync.dma_start(
                    out=xt_sb, in_=xT[b].rearrange("(o p) s -> p o s", p=128))

                # ---- qT (bf16, zero-padded by 2 cols each side) ----
                qt_sb = bigp.tile([128, NPAIR, SCW * SCH + 4], bf16, tag="qT")
                nc.vector.memset(qt_sb[:, :, 0:2], 0.0)
                nc.vector.memset(qt_sb[:, :, SCW * SCH + 2:], 0.0)
                for pr in range(NPAIR):
                    for n in range(SCH):
                        ps_q = psB.tile([128, SCW], f32, tag="ps512")
                        for dc in range(NDC):
                            nc.tensor.matmul(
                                ps_q,
                                wq_sb[:, dc, 128 * pr:128 * (pr + 1)],
                                xt_sb[:, dc, SCW * n:SCW * (n + 1)],
                                start=(dc == 0), stop=(dc == NDC - 1))
                        nc.scalar.copy(
                            out=qt_sb[:, pr, 2 + SCW * n:2 + SCW * (n + 1)],
                            in_=ps_q)

                # ---- k, v natural layout (bf16) ----
                knat = bigp.tile([128, NJ, D], bf16, tag="knat")
                vnat = bigp.tile([128, NJ, D], bf16, tag="vnat")
                for j in range(NJ):
                    for w_sb, dst in ((wk_sb, knat), (wv_sb, vnat)):
                        ps_k = psB.tile([128, D], f32, tag="ps512")
                        for dc in range(NDC):
                            nc.tensor.matmul(
                                ps_k,
                                xt_sb[:, dc, 128 * j:128 * (j + 1)],
                                w_sb[:, dc, :],
                                start=(dc == 0), stop=(dc == NDC - 1))
                        nc.vector.tensor_copy(out=dst[:, j, :], in_=ps_k)

                # ---- per-head cluster projections -> block-diag pair tiles ----
                kp_bd = bigp.tile([128, NPAIR, 128], bf16, tag="kpbd")
                vp_bd = bigp.tile([128, NPAIR, 128], bf16, tag="vpbd")
                nc.vector.memset(kp_bd, 0.0)
                nc.vector.memset(vp_bd, 0.0)
                for pr in range(NPAIR):
                    ae_sb = ahp.tile([128, NJ, 128], bf16, tag="ae")
                    af_sb = ahp.tile([128, NJ, 128], bf16, tag="af")
                    nc.sync.dma_start(out=ae_sb, in_=ae[pr].rearrange("(j p) c -> p j c", p=128))
                    nc.sync.dma_start(out=af_sb, in_=af[pr].rearrange("(j p) c -> p j c", p=128))
                    for a_sb, src, dst in ((ae_sb, knat, kp_bd), (af_sb, vnat, vp_bd)):
                        # lhsT = [A_h0 | A_h1] columns, rhs = both heads' k/v
                        # columns; out diag blocks = the two k_proj's, off-diag
                        # blocks are cross-head garbage and are not copied.
                        ps_p = psS.tile([128, 128], f32, tag="pssmall")
                        for j in range(NJ):
                            nc.tensor.matmul(
                                ps_p,
                                a_sb[:, j, :],
                                src[:, j, 128 * pr:128 * (pr + 1)],
                                start=(j == 0), stop=(j == NJ - 1))
                        nc.vector.tensor_copy(
                            out=dst[0:64, pr, 0:64], in_=ps_p[0:64, 0:64])
                        nc.vector.tensor_copy(
                            out=dst[64:128, pr, 64:128], in_=ps_p[64:128, 64:128])

                # ---- attention per head-pair ----
                concat_t = bigp.tile([128, NPAIR, S], bf16, tag="concatT")
                for pr in range(NPAIR):
                    # 5 tap matrices  BD_t = kp_bd^T(blkdiag) @ BDM_t
                    bdt = bdp.tile([128, 5, 128], bf16, tag="bdt")
                    for t in range(5):
                        ps_b = psS.tile([128, 128], f32, tag="pssmall")
                        nc.tensor.matmul(ps_b, kp_bd[:, pr, :], bdm_sb[:, t, :],
                                         start=True, stop=True)
                        nc.vector.tensor_copy(out=bdt[:, t, :], in_=ps_b)
                    for n in range(SCH):
                        ps_sc = psB.tile([128, SCW], f32, tag="ps512")
                        for ti in range(5):  # t = ti - 2
                            nc.tensor.matmul(
                                ps_sc,
                                bdt[:, ti, :],
                                qt_sb[:, pr, SCW * n + ti:SCW * n + ti + SCW],
                                start=(ti == 0), stop=(ti == 4))
                        expt = smp.tile([128, SCW], bf16, tag="expt")
                        nc.scalar.activation(
                            out=expt, in_=ps_sc,
                            func=mybir.ActivationFunctionType.Exp)
                        ps_at = psB.tile([128, SCW], f32, tag="ps512")
                        nc.tensor.matmul(ps_at, vp_bd[:, pr, :], expt,
                                         start=True, stop=True)
                        ps_z = psB.tile([128, SCW], f32, tag="ps512")
                        nc.tensor.matmul(ps_z, ones_sb, expt,
                                         start=True, stop=True)
                        rzb = smp.tile([128, SCW], f32, tag="rzb")
                        nc.vector.reciprocal(out=rzb, in_=ps_z)
                        nc.vector.tensor_mul(
                            out=concat_t[:, pr, SCW * n:SCW * (n + 1)],
                            in0=ps_at, in1=rzb)

                # ---- final dense + bias ----
                for j in range(NJ):
                    ps_d = psB.tile([128, D], f32, tag="ps512")
                    for dc in range(NDC):
                        nc.tensor.matmul(
                            ps_d,
                            concat_t[:, dc, 128 * j:128 * (j + 1)],
                            dw_sb[:, dc, :],
                            start=(dc == 0), stop=(dc == NDC - 1))
                    obuf = obp.tile([128, D], f32, tag="obuf")
                    nc.vector.tensor_add(out=obuf, in0=ps_d, in1=bias_sb)
                    nc.sync.dma_start(out=out[b, 128 * j:128 * (j + 1), :], in_=obuf)

    nc.finalize()
    return nc


def _prep_inputs(x, mask, wq, wk, wv, EW, FW, conv_w1, conv_w3, conv_w5, conv_b,
                 dense_w, dense_b, cluster_table):
    """Host-side restructuring -> per-core input maps."""
    bf = ml_dtypes.bfloat16
    x = np.ascontiguousarray(np.asarray(x, np.float32))
    mask = np.asarray(mask)
    counts = np.clip(mask.astype(np.int64).sum(1), 1, S)
    pos = np.asarray(cluster_table)[counts - 1]          # [B, P, C]
    if not (pos == pos[0]).all():
        raise NotImplementedError("per-batch cluster tables not supported")
    p0 = pos[0]                                          # [P, C]

    scale = 1.0 / np.sqrt(np.float32(DEPTH))
    s_idx = p0.ravel()
    c_idx = np.repeat(np.arange(P), C)

    def build_table(W, sc):
        A = np.zeros((H, S + 1, P), np.float32)
        np.add.at(A, (np.arange(H)[:, None], s_idx[None, :], c_idx[None, :]),
                  np.asarray(W, np.float32).reshape(H, P * C) * sc)
        return np.ascontiguousarray(A[:, :S, :])

    AE = build_table(EW, scale)
    AF = build_table(FW, 1.0)
    # pack adjacent heads side by side: [NPAIR, S, 128]
    AE = np.ascontiguousarray(
        AE.reshape(NPAIR, 2, S, P).transpose(0, 2, 1, 3).reshape(NPAIR, S, 128))
    AF = np.ascontiguousarray(
        AF.reshape(NPAIR, 2, S, P).transpose(0, 2, 1, 3).reshape(NPAIR, S, 128))

    # conv -> 5 tap matrices
    wp = np.arange(P)[:, None]
    jj = np.arange(P)[None, :]
    ii = wp - jj + 31
    valid = (ii >= 0) & (ii < P)
    ii = np.clip(ii, 0, P - 1)
    M = {t: np.zeros((P, P), np.float32) for t in range(-2, 3)}
    for cw, hk in ((conv_w1, 1), (conv_w3, 3), (conv_w5, 5)):
        cw = np.asarray(cw, np.float32)
        pad = (hk - 1) // 2
        for dy in range(hk):
            filt = cw[dy, :, 0, 0]
            M[dy - pad] += np.where(valid, filt[ii], 0.0) / 3.0
    BDM = np.zeros((5, 128, 128), np.float32)
    for ti in range(5):
        BDM[ti, :64, :64] = M[ti - 2]
        BDM[ti, 64:, 64:] = M[ti - 2]
    bbar = float(np.asarray(conv_b, np.float32).mean())
    if abs(bbar) > 1e-30:
        raise NotImplementedError("nonzero conv bias not folded")

    ones_bd = np.zeros((128, 128), np.float32)
    ones_bd[:64, :64] = 1.0
    ones_bd[64:, 64:] = 1.0

    # shard + transpose x
    xsh = x.reshape(NCORES, BLOC, S, D)
    in_maps = []
    shared = dict(
        wq=np.asarray(wq, np.float32), wk=np.asarray(wk, np.float32),
        wv=np.asarray(wv, np.float32),
        dw=np.asarray(dense_w, np.float32).astype(bf),
        db=np.asarray(dense_b, np.float32),
        ae=AE.astype(bf), af=AF.astype(bf), bdm=BDM.astype(bf),
        onesbd=ones_bd.astype(bf),
    )
    for c in range(NCORES):
        m = dict(shared)
        m["xT"] = np.ascontiguousarray(xsh[c].transpose(0, 2, 1))
        in_maps.append(m)
    return in_maps


def _run(in_maps, trace=False, tmpdir=None):
    from concourse.bass_utils import run_bass_kernel_spmd
    if "nc" not in _CACHE:
        _CACHE["nc"] = _build_nc()
    kw = {}
    if trace:
        _install_ntff_hook()
        kw = dict(trace=True, tmpdir=tmpdir)
    return run_bass_kernel_spmd(_CACHE["nc"], in_maps,
                                core_ids=list(range(NCORES)), **kw)


def _install_ntff_hook():
    import types, importlib.util as ilu
    if "antenv.axon_hooks" in sys.modules:
        return
    spec = ilu.spec_from_file_location(
        "trn_boot_mod", "/root/.axon_site/trn_agent_boot/trn_boot.py")
    tb = ilu.module_from_spec(spec)
    spec.loader.exec_module(tb)
    hook = tb._ntff_profile_via_ctypes("/opt/axon/libaxon_pjrt.so")
    mod = types.ModuleType("antenv.axon_hooks")
    mod.get_axon_ntff_profile_hook = lambda: hook
    import antenv  # noqa: F401
    sys.modules["antenv.axon_hooks"] = mod


def kernel(**inputs) -> np.ndarray:
    in_maps = _prep_inputs(**inputs)
    r = _run(in_maps)
    return np.concatenate([r.results[c]["out"] for c in range(NCORES)], axis=0)


# revision 8
# speedup vs baseline: 1.0572x; 1.0572x over previous
"""Clustered Linformer Attention — Trainium2 Bass kernel, 8 NeuronCores.

Strategy: data-parallel over batch (2 batches/core, no collectives).
Math restructuring (verified vs reference to ~7e-7 in f32):
  - mask is all-ones => cluster c holds positions [32c, 32c+32); the per-head
    gather+einsum projections become  k_proj = AE[h]^T @ k_h  with a host-built
    sparse table AE[h] in [S, P] (score scale folded in), same for v with AF.
  - the 3-kernel conv fusion over scores collapses to 5 "tap" matrices M_t in
    [P, P] (t in -2..2):  scores_conv[s] = sum_t  (q[s+t] @ (k_proj^T @ M_t)).
    Taps are applied as 5 PSUM-accumulated matmuls with a column-shifted
    (zero-padded) q^T operand.
  - adjacent heads are packed block-diagonally so every matmul contracts over
    the full 128 partitions.
  - softmax has no max-subtraction (|scores| <~ 1.6, exp is safe in f32);
    Z = sum_c exp is computed by an all-ones block-diag matmul that also
    broadcasts Z to all 128 partitions, so normalization is one DVE op.
"""
import sys
import numpy as np
import ml_dtypes

sys.path.insert(0, '/opt/trn_rl_repo')

B, S, D = 16, 2048, 512
H, P, C = 8, 64, 32
DEPTH = D // H           # 64
NCORES = 8
BLOC = B // NCORES       # 2 batches per core
NPAIR = H // 2           # 4 head pairs
SCH = 4                  # s-chunks of 512
SCW = S // SCH           # 512
NJ = S // 128            # 16 s-tiles of 128
NDC = D // 128           # 4 contraction chunks

_CACHE = {}


def _build_nc():
    import concourse.tile as tile
    from concourse import mybir, bacc

    f32 = mybir.dt.float32
    f32r = mybir.dt.float32r
    bf16 = mybir.dt.bfloat16

    nc = bacc.Bacc()
    xT = nc.declare_dram_parameter("xT", [BLOC, D, S], f32r, isOutput=False)
    wq = nc.declare_dram_parameter("wq", [D, D], f32r, isOutput=False)
    wk = nc.declare_dram_parameter("wk", [D, D], f32r, isOutput=False)
    wv = nc.declare_dram_parameter("wv", [D, D], f32r, isOutput=False)
    dw = nc.declare_dram_parameter("dw", [D, D], bf16, isOutput=False)
    db = nc.declare_dram_parameter("db", [D], f32, isOutput=False)
    ae = nc.declare_dram_parameter("ae", [NPAIR, S, 128], bf16, isOutput=False)
    af = nc.declare_dram_parameter("af", [NPAIR, S, 128], bf16, isOutput=False)
    bdm = nc.declare_dram_parameter("bdm", [5, 128, 128], bf16, isOutput=False)
    onesbd = nc.declare_dram_parameter("onesbd", [128, 128], bf16, isOutput=False)
    out = nc.declare_dram_parameter("out", [BLOC, S, D], f32, isOutput=True)

    import concourse.bass as bass

    with tile.TileContext(nc) as tc:
        with tc.tile_pool(name="const", bufs=1) as cpool, \
             tc.tile_pool(name="big", bufs=1) as bigp, \
             tc.tile_pool(name="ah", bufs=2) as ahp, \
             tc.tile_pool(name="sm", bufs=4) as smp, \
             tc.tile_pool(name="bd", bufs=2) as bdp, \
             tc.tile_pool(name="ob", bufs=3) as obp, \
             tc.tile_pool(name="psB", bufs=5, space="PSUM") as psB, \
             tc.tile_pool(name="psS", bufs=2, space="PSUM") as psS:

            # ---- constants in SBUF ----
            wq_sb = cpool.tile([128, NDC, D], f32r)
            wk_sb = cpool.tile([128, NDC, D], f32r)
            wv_sb = cpool.tile([128, NDC, D], f32r)
            dw_sb = cpool.tile([128, NDC, D], bf16)
            for t_sb, t_dr in ((wq_sb, wq), (wk_sb, wk), (wv_sb, wv), (dw_sb, dw)):
                nc.sync.dma_start(out=t_sb, in_=t_dr[:].rearrange("(o p) m -> p o m", p=128))
            bdm_sb = cpool.tile([128, 5, 128], bf16)
            nc.sync.dma_start(out=bdm_sb, in_=bdm[:].rearrange("t p m -> p t m"))
            ones_sb = cpool.tile([128, 128], bf16)
            nc.sync.dma_start(out=ones_sb, in_=onesbd[:])
            bias_sb = cpool.tile([128, D], f32)
            db_ap = db[:]
            nc.gpsimd.dma_start(
                out=bias_sb,
                in_=bass.AP(tensor=db_ap.tensor, offset=db_ap.offset,
                            ap=[[0, 128]] + list(db_ap.ap)))

            for b in range(BLOC):
                # ---- load pre-transposed x (4 rotating chunk tiles) ----
                xt_sb = [bigp.tile([128, S], f32r, tag="xt", bufs=4,
                                   name=f"xt_{b}_{dc}")
                         for dc in range(NDC)]
                for dc in range(NDC):
                    nc.sync.dma_start(
                        out=xt_sb[dc],
                        in_=xT[b, 128 * dc:128 * (dc + 1), :])

                # ---- k, v natural layout (bf16) ----
                knat = bigp.tile([128, NJ, D], bf16, tag="knat")
                vnat = bigp.tile([128, NJ, D], bf16, tag="vnat")
                for j in range(NJ):
                    for w_sb, dst in ((wk_sb, knat), (wv_sb, vnat)):
                        ps_k = psB.tile([128, D], f32, tag="ps512")
                        for dc in range(NDC):
                            nc.tensor.matmul(
                                ps_k,
                                xt_sb[dc][:, 128 * j:128 * (j + 1)],
                                w_sb[:, dc, :],
                                start=(dc == 0), stop=(dc == NDC - 1))
                        nc.vector.tensor_copy(out=dst[:, j, :], in_=ps_k)

                # ---- qT (bf16, zero-padded by 2 cols each side) ----
                qt_sb = bigp.tile([128, NPAIR, SCW * SCH + 4], bf16, tag="qT")
                nc.vector.memset(qt_sb[:, :, 0:2], 0.0)
                nc.vector.memset(qt_sb[:, :, SCW * SCH + 2:], 0.0)
                for pr in range(NPAIR):
                    for n in range(SCH):
                        ps_q = psB.tile([128, SCW], f32, tag="ps512")
                        for dc in range(NDC):
                            nc.tensor.matmul(
                                ps_q,
                                wq_sb[:, dc, 128 * pr:128 * (pr + 1)],
                                xt_sb[dc][:, SCW * n:SCW * (n + 1)],
                                start=(dc == 0), stop=(dc == NDC - 1))
                        nc.scalar.copy(
                            out=qt_sb[:, pr, 2 + SCW * n:2 + SCW * (n + 1)],
                            in_=ps_q)

                # ---- per-head cluster projections -> block-diag pair tiles ----
                kp_bd = bigp.tile([128, NPAIR, 128], bf16, tag="kpbd")
                vp_bd = bigp.tile([128, NPAIR, 128], bf16, tag="vpbd")
                nc.vector.memset(kp_bd, 0.0)
                nc.vector.memset(vp_bd, 0.0)
                for pr in range(NPAIR):
                    ae_sb = ahp.tile([128, NJ, 128], bf16, tag="ae")
                    af_sb = ahp.tile([128, NJ, 128], bf16, tag="af")
                    nc.sync.dma_start(out=ae_sb, in_=ae[pr].rearrange("(j p) c -> p j c", p=128))
                    nc.sync.dma_start(out=af_sb, in_=af[pr].rearrange("(j p) c -> p j c", p=128))
                    for a_sb, src, dst in ((ae_sb, knat, kp_bd), (af_sb, vnat, vp_bd)):
                        # lhsT = [A_h0 | A_h1] columns, rhs = both heads' k/v
                        # columns; out diag blocks = the two k_proj's, off-diag
                        # blocks are cross-head garbage and are not copied.
                        ps_p = psS.tile([128, 128], f32, tag="pssmall")
                        for j in range(NJ):
                            nc.tensor.matmul(
                                ps_p,
                                a_sb[:, j, :],
                                src[:, j, 128 * pr:128 * (pr + 1)],
                                start=(j == 0), stop=(j == NJ - 1))
                        nc.vector.tensor_copy(
                            out=dst[0:64, pr, 0:64], in_=ps_p[0:64, 0:64])
                        nc.vector.tensor_copy(
                            out=dst[64:128, pr, 64:128], in_=ps_p[64:128, 64:128])

                # ---- attention per head-pair ----
                concat_t = bigp.tile([128, NPAIR, S], bf16, tag="concatT")
                for pr in range(NPAIR):
                    # 5 tap matrices  BD_t = kp_bd^T(blkdiag) @ BDM_t
                    bdt = bdp.tile([128, 5, 128], bf16, tag="bdt")
                    for t in range(5):
                        ps_b = psS.tile([128, 128], f32, tag="pssmall")
                        nc.tensor.matmul(ps_b, kp_bd[:, pr, :], bdm_sb[:, t, :],
                                         start=True, stop=True)
                        nc.vector.tensor_copy(out=bdt[:, t, :], in_=ps_b)
                    for n in range(SCH):
                        ps_sc = psB.tile([128, SCW], f32, tag="ps512")
                        for ti in range(5):  # t = ti - 2
                            nc.tensor.matmul(
                                ps_sc,
                                bdt[:, ti, :],
                                qt_sb[:, pr, SCW * n + ti:SCW * n + ti + SCW],
                                start=(ti == 0), stop=(ti == 4))
                        expt = smp.tile([128, SCW], bf16, tag="expt")
                        nc.scalar.activation(
                            out=expt, in_=ps_sc,
                            func=mybir.ActivationFunctionType.Exp)
                        ps_at = psB.tile([128, SCW], f32, tag="ps512")
                        nc.tensor.matmul(ps_at, vp_bd[:, pr, :], expt,
                                         start=True, stop=True)
                        ps_z = psB.tile([128, SCW], f32, tag="ps512")
                        nc.tensor.matmul(ps_z, ones_sb, expt,
                                         start=True, stop=True)
                        # 1/Z via exp(-ln(Z)) on ACT: DVE reciprocal is ~3.3us
                        # per tile and stalls the whole normalize->dense chain.
                        lnz = smp.tile([128, SCW], f32, tag="lnz")
                        nc.scalar.activation(
                            out=lnz, in_=ps_z,
                            func=mybir.ActivationFunctionType.Ln)
                        rzb = smp.tile([128, SCW], f32, tag="rzb")
                        nc.scalar.activation(
                            out=rzb, in_=lnz,
                            func=mybir.ActivationFunctionType.Exp, scale=-1.0)
                        nc.vector.tensor_mul(
                            out=concat_t[:, pr, SCW * n:SCW * (n + 1)],
                            in0=ps_at, in1=rzb)

                # ---- final dense + bias ----
                for j in range(NJ):
                    ps_d = psB.tile([128, D], f32, tag="ps512")
                    for dc in range(NDC):
                        nc.tensor.matmul(
                            ps_d,
                            concat_t[:, dc, 128 * j:128 * (j + 1)],
                            dw_sb[:, dc, :],
                            start=(dc == 0), stop=(dc == NDC - 1))
                    obuf = obp.tile([128, D], f32, tag="obuf")
                    nc.vector.tensor_add(out=obuf, in0=ps_d, in1=bias_sb)
                    nc.sync.dma_start(out=out[b, 128 * j:128 * (j + 1), :], in_=obuf)

    nc.finalize()
    return nc


def _prep_inputs(x, mask, wq, wk, wv, EW, FW, conv_w1, conv_w3, conv_w5, conv_b,
                 dense_w, dense_b, cluster_table):
    """Host-side restructuring -> per-core input maps."""
    bf = ml_dtypes.bfloat16
    x = np.ascontiguousarray(np.asarray(x, np.float32))
    mask = np.asarray(mask)
    counts = np.clip(mask.astype(np.int64).sum(1), 1, S)
    pos = np.asarray(cluster_table)[counts - 1]          # [B, P, C]
    if not (pos == pos[0]).all():
        raise NotImplementedError("per-batch cluster tables not supported")
    p0 = pos[0]                                          # [P, C]

    scale = 1.0 / np.sqrt(np.float32(DEPTH))
    s_idx = p0.ravel()
    c_idx = np.repeat(np.arange(P), C)

    def build_table(W, sc):
        A = np.zeros((H, S + 1, P), np.float32)
        np.add.at(A, (np.arange(H)[:, None], s_idx[None, :], c_idx[None, :]),
                  np.asarray(W, np.float32).reshape(H, P * C) * sc)
        return np.ascontiguousarray(A[:, :S, :])

    AE = build_table(EW, scale)
    AF = build_table(FW, 1.0)
    # pack adjacent heads side by side: [NPAIR, S, 128]
    AE = np.ascontiguousarray(
        AE.reshape(NPAIR, 2, S, P).transpose(0, 2, 1, 3).reshape(NPAIR, S, 128))
    AF = np.ascontiguousarray(
        AF.reshape(NPAIR, 2, S, P).transpose(0, 2, 1, 3).reshape(NPAIR, S, 128))

    # conv -> 5 tap matrices
    wp = np.arange(P)[:, None]
    jj = np.arange(P)[None, :]
    ii = wp - jj + 31
    valid = (ii >= 0) & (ii < P)
    ii = np.clip(ii, 0, P - 1)
    M = {t: np.zeros((P, P), np.float32) for t in range(-2, 3)}
    for cw, hk in ((conv_w1, 1), (conv_w3, 3), (conv_w5, 5)):
        cw = np.asarray(cw, np.float32)
        pad = (hk - 1) // 2
        for dy in range(hk):
            filt = cw[dy, :, 0, 0]
            M[dy - pad] += np.where(valid, filt[ii], 0.0) / 3.0
    BDM = np.zeros((5, 128, 128), np.float32)
    for ti in range(5):
        BDM[ti, :64, :64] = M[ti - 2]
        BDM[ti, 64:, 64:] = M[ti - 2]
    bbar = float(np.asarray(conv_b, np.float32).mean())
    if abs(bbar) > 1e-30:
        raise NotImplementedError("nonzero conv bias not folded")

    ones_bd = np.zeros((128, 128), np.float32)
    ones_bd[:64, :64] = 1.0
    ones_bd[64:, 64:] = 1.0

    # shard + transpose x
    xsh = x.reshape(NCORES, BLOC, S, D)
    in_maps = []
    shared = dict(
        wq=np.asarray(wq, np.float32), wk=np.asarray(wk, np.float32),
        wv=np.asarray(wv, np.float32),
        dw=np.asarray(dense_w, np.float32).astype(bf),
        db=np.asarray(dense_b, np.float32),
        ae=AE.astype(bf), af=AF.astype(bf), bdm=BDM.astype(bf),
        onesbd=ones_bd.astype(bf),
    )
    for c in range(NCORES):
        m = dict(shared)
        m["xT"] = np.ascontiguousarray(xsh[c].transpose(0, 2, 1))
        in_maps.append(m)
    return in_maps


def _run(in_maps, trace=False, tmpdir=None):
    from concourse.bass_utils import run_bass_kernel_spmd
    if "nc" not in _CACHE:
        _CACHE["nc"] = _build_nc()
    kw = {}
    if trace:
        _install_ntff_hook()
        kw = dict(trace=True, tmpdir=tmpdir)
    return run_bass_kernel_spmd(_CACHE["nc"], in_maps,
                                core_ids=list(range(NCORES)), **kw)


def _install_ntff_hook():
    import types, importlib.util as ilu
    if "antenv.axon_hooks" in sys.modules:
        return
    spec = ilu.spec_from_file_location(
        "trn_boot_mod", "/root/.axon_site/trn_agent_boot/trn_boot.py")
    tb = ilu.module_from_spec(spec)
    spec.loader.exec_module(tb)
    hook = tb._ntff_profile_via_ctypes("/opt/axon/libaxon_pjrt.so")
    mod = types.ModuleType("antenv.axon_hooks")
    mod.get_axon_ntff_profile_hook = lambda: hook
    import antenv  # noqa: F401
    sys.modules["antenv.axon_hooks"] = mod


def kernel(**inputs) -> np.ndarray:
    in_maps = _prep_inputs(**inputs)
    r = _run(in_maps)
    return np.concatenate([r.results[c]["out"] for c in range(NCORES)], axis=0)


# revision 9
# speedup vs baseline: 1.3583x; 1.2848x over previous
"""Clustered Linformer Attention — Trainium2 Bass kernel, 8 NeuronCores.

Strategy: data-parallel over batch (2 batches/core, no collectives).
Math restructuring (verified vs reference to ~7e-7 in f32):
  - mask is all-ones => cluster c holds positions [32c, 32c+32); the per-head
    gather+einsum projections become  k_proj = AE[h]^T @ k_h  with a host-built
    sparse table AE[h] in [S, P] (score scale folded in), same for v with AF.
  - the 3-kernel conv fusion over scores collapses to 5 "tap" matrices M_t in
    [P, P] (t in -2..2):  scores_conv[s] = sum_t  (q[s+t] @ (k_proj^T @ M_t)).
    Taps are applied as 5 PSUM-accumulated matmuls with a column-shifted
    (zero-padded) q^T operand.
  - adjacent heads are packed block-diagonally so every matmul contracts over
    the full 128 partitions.
  - softmax has no max-subtraction (|scores| <~ 1.6, exp is safe in f32);
    Z = sum_c exp is computed by an all-ones block-diag matmul that also
    broadcasts Z to all 128 partitions, so normalization is one DVE op.
"""
import sys
import numpy as np
import ml_dtypes

sys.path.insert(0, '/opt/trn_rl_repo')

B, S, D = 16, 2048, 512
H, P, C = 8, 64, 32
DEPTH = D // H           # 64
NCORES = 8
BLOC = B // NCORES       # 2 batches per core
NPAIR = H // 2           # 4 head pairs
SCH = 4                  # s-chunks of 512
SCW = S // SCH           # 512
NJ = S // 128            # 16 s-tiles of 128
NDC = D // 128           # 4 contraction chunks

_CACHE = {}


def _build_nc():
    import concourse.tile as tile
    from concourse import mybir, bacc

    f32 = mybir.dt.float32
    f32r = mybir.dt.float32r
    bf16 = mybir.dt.bfloat16

    nc = bacc.Bacc()
    xT = nc.declare_dram_parameter("xT", [BLOC, D, S], bf16, isOutput=False)
    wq = nc.declare_dram_parameter("wq", [D, D], bf16, isOutput=False)
    wk = nc.declare_dram_parameter("wk", [D, D], bf16, isOutput=False)
    wv = nc.declare_dram_parameter("wv", [D, D], bf16, isOutput=False)
    dw = nc.declare_dram_parameter("dw", [D, D], bf16, isOutput=False)
    db = nc.declare_dram_parameter("db", [D], f32, isOutput=False)
    ae = nc.declare_dram_parameter("ae", [NPAIR, S, 128], bf16, isOutput=False)
    af = nc.declare_dram_parameter("af", [NPAIR, S, 128], bf16, isOutput=False)
    bdm = nc.declare_dram_parameter("bdm", [5, 128, 128], bf16, isOutput=False)
    onesbd = nc.declare_dram_parameter("onesbd", [128, 128], bf16, isOutput=False)
    out = nc.declare_dram_parameter("out", [BLOC, S, D], f32, isOutput=True)

    import concourse.bass as bass

    with tile.TileContext(nc) as tc:
        with tc.tile_pool(name="const", bufs=1) as cpool, \
             tc.tile_pool(name="big", bufs=1) as bigp, \
             tc.tile_pool(name="ah", bufs=2) as ahp, \
             tc.tile_pool(name="sm", bufs=4) as smp, \
             tc.tile_pool(name="bd", bufs=2) as bdp, \
             tc.tile_pool(name="ob", bufs=3) as obp, \
             tc.tile_pool(name="psB", bufs=5, space="PSUM") as psB, \
             tc.tile_pool(name="psS", bufs=2, space="PSUM") as psS:

            # ---- constants in SBUF ----
            wq_sb = cpool.tile([128, NDC, D], bf16)
            wk_sb = cpool.tile([128, NDC, D], bf16)
            wv_sb = cpool.tile([128, NDC, D], bf16)
            dw_sb = cpool.tile([128, NDC, D], bf16)
            for t_sb, t_dr in ((wq_sb, wq), (wk_sb, wk), (wv_sb, wv), (dw_sb, dw)):
                nc.sync.dma_start(out=t_sb, in_=t_dr[:].rearrange("(o p) m -> p o m", p=128))
            bdm_sb = cpool.tile([128, 5, 128], bf16)
            nc.sync.dma_start(out=bdm_sb, in_=bdm[:].rearrange("t p m -> p t m"))
            ones_sb = cpool.tile([128, 128], bf16)
            nc.sync.dma_start(out=ones_sb, in_=onesbd[:])
            bias_sb = cpool.tile([128, D], f32)
            db_ap = db[:]
            nc.gpsimd.dma_start(
                out=bias_sb,
                in_=bass.AP(tensor=db_ap.tensor, offset=db_ap.offset,
                            ap=[[0, 128]] + list(db_ap.ap)))

            for b in range(BLOC):
                # ---- load pre-transposed x (4 rotating chunk tiles) ----
                xt_sb = [bigp.tile([128, S], bf16, tag="xt", bufs=4,
                                   name=f"xt_{b}_{dc}")
                         for dc in range(NDC)]
                for dc in range(NDC):
                    nc.sync.dma_start(
                        out=xt_sb[dc],
                        in_=xT[b, 128 * dc:128 * (dc + 1), :])

                # ---- k, v natural layout (bf16) ----
                knat = bigp.tile([128, NJ, D], bf16, tag="knat")
                vnat = bigp.tile([128, NJ, D], bf16, tag="vnat")
                for j in range(NJ):
                    for w_sb, dst in ((wk_sb, knat), (wv_sb, vnat)):
                        ps_k = psB.tile([128, D], f32, tag="ps512")
                        for dc in range(NDC):
                            nc.tensor.matmul(
                                ps_k,
                                xt_sb[dc][:, 128 * j:128 * (j + 1)],
                                w_sb[:, dc, :],
                                start=(dc == 0), stop=(dc == NDC - 1))
                        nc.vector.tensor_copy(out=dst[:, j, :], in_=ps_k)

                # ---- qT (bf16, zero-padded by 2 cols each side) ----
                qt_sb = bigp.tile([128, NPAIR, SCW * SCH + 4], bf16, tag="qT")
                nc.vector.memset(qt_sb[:, :, 0:2], 0.0)
                nc.vector.memset(qt_sb[:, :, SCW * SCH + 2:], 0.0)
                for pr in range(NPAIR):
                    for n in range(SCH):
                        ps_q = psB.tile([128, SCW], f32, tag="ps512")
                        for dc in range(NDC):
                            nc.tensor.matmul(
                                ps_q,
                                wq_sb[:, dc, 128 * pr:128 * (pr + 1)],
                                xt_sb[dc][:, SCW * n:SCW * (n + 1)],
                                start=(dc == 0), stop=(dc == NDC - 1))
                        nc.scalar.copy(
                            out=qt_sb[:, pr, 2 + SCW * n:2 + SCW * (n + 1)],
                            in_=ps_q)

                # ---- per-head cluster projections -> block-diag pair tiles ----
                kp_bd = bigp.tile([128, NPAIR, 128], bf16, tag="kpbd")
                vp_bd = bigp.tile([128, NPAIR, 128], bf16, tag="vpbd")
                nc.vector.memset(kp_bd, 0.0)
                nc.vector.memset(vp_bd, 0.0)
                for pr in range(NPAIR):
                    ae_sb = ahp.tile([128, NJ, 128], bf16, tag="ae")
                    af_sb = ahp.tile([128, NJ, 128], bf16, tag="af")
                    nc.sync.dma_start(out=ae_sb, in_=ae[pr].rearrange("(j p) c -> p j c", p=128))
                    nc.sync.dma_start(out=af_sb, in_=af[pr].rearrange("(j p) c -> p j c", p=128))
                    for a_sb, src, dst in ((ae_sb, knat, kp_bd), (af_sb, vnat, vp_bd)):
                        # lhsT = [A_h0 | A_h1] columns, rhs = both heads' k/v
                        # columns; out diag blocks = the two k_proj's, off-diag
                        # blocks are cross-head garbage and are not copied.
                        ps_p = psS.tile([128, 128], f32, tag="pssmall")
                        for j in range(NJ):
                            nc.tensor.matmul(
                                ps_p,
                                a_sb[:, j, :],
                                src[:, j, 128 * pr:128 * (pr + 1)],
                                start=(j == 0), stop=(j == NJ - 1))
                        nc.vector.tensor_copy(
                            out=dst[0:64, pr, 0:64], in_=ps_p[0:64, 0:64])
                        nc.vector.tensor_copy(
                            out=dst[64:128, pr, 64:128], in_=ps_p[64:128, 64:128])

                # ---- attention per head-pair ----
                concat_t = bigp.tile([128, NPAIR, S], bf16, tag="concatT")
                for pr in range(NPAIR):
                    # 5 tap matrices  BD_t = kp_bd^T(blkdiag) @ BDM_t
                    bdt = bdp.tile([128, 5, 128], bf16, tag="bdt")
                    for t in range(5):
                        ps_b = psS.tile([128, 128], f32, tag="pssmall")
                        nc.tensor.matmul(ps_b, kp_bd[:, pr, :], bdm_sb[:, t, :],
                                         start=True, stop=True)
                        nc.vector.tensor_copy(out=bdt[:, t, :], in_=ps_b)
                    for n in range(SCH):
                        ps_sc = psB.tile([128, SCW], f32, tag="ps512")
                        for ti in range(5):  # t = ti - 2
                            nc.tensor.matmul(
                                ps_sc,
                                bdt[:, ti, :],
                                qt_sb[:, pr, SCW * n + ti:SCW * n + ti + SCW],
                                start=(ti == 0), stop=(ti == 4))
                        expt = smp.tile([128, SCW], bf16, tag="expt")
                        nc.scalar.activation(
                            out=expt, in_=ps_sc,
                            func=mybir.ActivationFunctionType.Exp)
                        ps_at = psB.tile([128, SCW], f32, tag="ps512")
                        nc.tensor.matmul(ps_at, vp_bd[:, pr, :], expt,
                                         start=True, stop=True)
                        ps_z = psB.tile([128, SCW], f32, tag="ps512")
                        nc.tensor.matmul(ps_z, ones_sb, expt,
                                         start=True, stop=True)
                        # 1/Z: approx reciprocal (~18 bits, single DVE op).
                        # Exact reciprocal is ~3.3us/tile; ACT ln/exp thrashes
                        # the activation table (1.3us reload per switch).
                        rzb = smp.tile([128, SCW], f32, tag="rzb")
                        nc.vector.reciprocal_approx_fast(out=rzb, in_=ps_z)
                        nc.vector.tensor_mul(
                            out=concat_t[:, pr, SCW * n:SCW * (n + 1)],
                            in0=ps_at, in1=rzb)

                # ---- final dense + bias ----
                for j in range(NJ):
                    ps_d = psB.tile([128, D], f32, tag="ps512")
                    for dc in range(NDC):
                        nc.tensor.matmul(
                            ps_d,
                            concat_t[:, dc, 128 * j:128 * (j + 1)],
                            dw_sb[:, dc, :],
                            start=(dc == 0), stop=(dc == NDC - 1))
                    obuf = obp.tile([128, D], f32, tag="obuf")
                    nc.vector.tensor_add(out=obuf, in0=ps_d, in1=bias_sb)
                    nc.sync.dma_start(out=out[b, 128 * j:128 * (j + 1), :], in_=obuf)

    nc.finalize()
    return nc


def _prep_inputs(x, mask, wq, wk, wv, EW, FW, conv_w1, conv_w3, conv_w5, conv_b,
                 dense_w, dense_b, cluster_table):
    """Host-side restructuring -> per-core input maps."""
    bf = ml_dtypes.bfloat16
    x = np.ascontiguousarray(np.asarray(x, np.float32))
    mask = np.asarray(mask)
    counts = np.clip(mask.astype(np.int64).sum(1), 1, S)
    pos = np.asarray(cluster_table)[counts - 1]          # [B, P, C]
    if not (pos == pos[0]).all():
        raise NotImplementedError("per-batch cluster tables not supported")
    p0 = pos[0]                                          # [P, C]

    scale = 1.0 / np.sqrt(np.float32(DEPTH))
    s_idx = p0.ravel()
    c_idx = np.repeat(np.arange(P), C)

    def build_table(W, sc):
        A = np.zeros((H, S + 1, P), np.float32)
        np.add.at(A, (np.arange(H)[:, None], s_idx[None, :], c_idx[None, :]),
                  np.asarray(W, np.float32).reshape(H, P * C) * sc)
        return np.ascontiguousarray(A[:, :S, :])

    AE = build_table(EW, scale)
    AF = build_table(FW, 1.0)
    # pack adjacent heads side by side: [NPAIR, S, 128]
    AE = np.ascontiguousarray(
        AE.reshape(NPAIR, 2, S, P).transpose(0, 2, 1, 3).reshape(NPAIR, S, 128))
    AF = np.ascontiguousarray(
        AF.reshape(NPAIR, 2, S, P).transpose(0, 2, 1, 3).reshape(NPAIR, S, 128))

    # conv -> 5 tap matrices
    wp = np.arange(P)[:, None]
    jj = np.arange(P)[None, :]
    ii = wp - jj + 31
    valid = (ii >= 0) & (ii < P)
    ii = np.clip(ii, 0, P - 1)
    M = {t: np.zeros((P, P), np.float32) for t in range(-2, 3)}
    for cw, hk in ((conv_w1, 1), (conv_w3, 3), (conv_w5, 5)):
        cw = np.asarray(cw, np.float32)
        pad = (hk - 1) // 2
        for dy in range(hk):
            filt = cw[dy, :, 0, 0]
            M[dy - pad] += np.where(valid, filt[ii], 0.0) / 3.0
    BDM = np.zeros((5, 128, 128), np.float32)
    for ti in range(5):
        BDM[ti, :64, :64] = M[ti - 2]
        BDM[ti, 64:, 64:] = M[ti - 2]
    bbar = float(np.asarray(conv_b, np.float32).mean())
    if abs(bbar) > 1e-30:
        raise NotImplementedError("nonzero conv bias not folded")

    ones_bd = np.zeros((128, 128), np.float32)
    ones_bd[:64, :64] = 1.0
    ones_bd[64:, 64:] = 1.0

    # shard + transpose x
    xsh = x.reshape(NCORES, BLOC, S, D)
    in_maps = []
    shared = dict(
        wq=np.asarray(wq, np.float32).astype(bf),
        wk=np.asarray(wk, np.float32).astype(bf),
        wv=np.asarray(wv, np.float32).astype(bf),
        dw=np.asarray(dense_w, np.float32).astype(bf),
        db=np.asarray(dense_b, np.float32),
        ae=AE.astype(bf), af=AF.astype(bf), bdm=BDM.astype(bf),
        onesbd=ones_bd.astype(bf),
    )
    for c in range(NCORES):
        m = dict(shared)
        m["xT"] = np.ascontiguousarray(xsh[c].transpose(0, 2, 1)).astype(bf)
        in_maps.append(m)
    return in_maps


def _run(in_maps, trace=False, tmpdir=None):
    from concourse.bass_utils import run_bass_kernel_spmd
    if "nc" not in _CACHE:
        _CACHE["nc"] = _build_nc()
    kw = {}
    if trace:
        _install_ntff_hook()
        kw = dict(trace=True, tmpdir=tmpdir)
    return run_bass_kernel_spmd(_CACHE["nc"], in_maps,
                                core_ids=list(range(NCORES)), **kw)


def _install_ntff_hook():
    import types, importlib.util as ilu
    if "antenv.axon_hooks" in sys.modules:
        return
    spec = ilu.spec_from_file_location(
        "trn_boot_mod", "/root/.axon_site/trn_agent_boot/trn_boot.py")
    tb = ilu.module_from_spec(spec)
    spec.loader.exec_module(tb)
    hook = tb._ntff_profile_via_ctypes("/opt/axon/libaxon_pjrt.so")
    mod = types.ModuleType("antenv.axon_hooks")
    mod.get_axon_ntff_profile_hook = lambda: hook
    import antenv  # noqa: F401
    sys.modules["antenv.axon_hooks"] = mod


def kernel(**inputs) -> np.ndarray:
    in_maps = _prep_inputs(**inputs)
    r = _run(in_maps)
    return np.concatenate([r.results[c]["out"] for c in range(NCORES)], axis=0)


# revision 11
# speedup vs baseline: 1.4664x; 1.0796x over previous
"""Clustered Linformer Attention — Trainium2 Bass kernel, 8 NeuronCores.

Strategy: data-parallel over batch (2 batches/core, no collectives).
Math restructuring (verified vs reference to ~7e-7 in f32):
  - mask is all-ones => cluster c holds positions [32c, 32c+32); the per-head
    gather+einsum projections become  k_proj = AE[h]^T @ k_h  with a host-built
    sparse table AE[h] in [S, P] (score scale folded in), same for v with AF.
  - the 3-kernel conv fusion over scores collapses to 5 "tap" matrices M_t in
    [P, P] (t in -2..2):  scores_conv[s] = sum_t  (q[s+t] @ (k_proj^T @ M_t)).
    Taps are applied as 5 PSUM-accumulated matmuls with a column-shifted
    (zero-padded) q^T operand.
  - adjacent heads are packed block-diagonally so every matmul contracts over
    the full 128 partitions.
  - softmax has no max-subtraction (|scores| <~ 1.6, exp is safe in f32);
    Z = sum_c exp is computed by an all-ones block-diag matmul that also
    broadcasts Z to all 128 partitions, so normalization is one DVE op.
"""
import sys
import numpy as np
import ml_dtypes

sys.path.insert(0, '/opt/trn_rl_repo')

B, S, D = 16, 2048, 512
H, P, C = 8, 64, 32
DEPTH = D // H           # 64
NCORES = 8
BLOC = B // NCORES       # 2 batches per core
NPAIR = H // 2           # 4 head pairs
SCH = 4                  # s-chunks of 512
SCW = S // SCH           # 512
NJ = S // 128            # 16 s-tiles of 128
NDC = D // 128           # 4 contraction chunks

_CACHE = {}


def _build_nc():
    import concourse.tile as tile
    from concourse import mybir, bacc

    f32 = mybir.dt.float32
    f32r = mybir.dt.float32r
    bf16 = mybir.dt.bfloat16

    nc = bacc.Bacc()
    xT = nc.declare_dram_parameter("xT", [BLOC, D, S], bf16, isOutput=False)
    wq = nc.declare_dram_parameter("wq", [D, D], bf16, isOutput=False)
    wk = nc.declare_dram_parameter("wk", [D, D], bf16, isOutput=False)
    wv = nc.declare_dram_parameter("wv", [D, D], bf16, isOutput=False)
    dw = nc.declare_dram_parameter("dw", [D, D], bf16, isOutput=False)
    db = nc.declare_dram_parameter("db", [D], f32, isOutput=False)
    ae = nc.declare_dram_parameter("ae", [NPAIR, S, 128], bf16, isOutput=False)
    af = nc.declare_dram_parameter("af", [NPAIR, S, 128], bf16, isOutput=False)
    bdm = nc.declare_dram_parameter("bdm", [5, 128, 128], bf16, isOutput=False)
    onesbd = nc.declare_dram_parameter("onesbd", [128, 128], bf16, isOutput=False)
    out = nc.declare_dram_parameter("out", [BLOC, S, D], f32, isOutput=True)

    import concourse.bass as bass

    with tile.TileContext(nc) as tc:
        with tc.tile_pool(name="const", bufs=1) as cpool, \
             tc.tile_pool(name="big", bufs=1) as bigp, \
             tc.tile_pool(name="ah", bufs=2) as ahp, \
             tc.tile_pool(name="sm", bufs=4) as smp, \
             tc.tile_pool(name="bd", bufs=2) as bdp, \
             tc.tile_pool(name="ob", bufs=3) as obp, \
             tc.tile_pool(name="psB", bufs=6, space="PSUM") as psB, \
             tc.tile_pool(name="psS", bufs=2, space="PSUM") as psS:

            # ---- constants in SBUF ----
            wq_sb = cpool.tile([128, NDC, D], bf16)
            wk_sb = cpool.tile([128, NDC, D], bf16)
            wv_sb = cpool.tile([128, NDC, D], bf16)
            dw_sb = cpool.tile([128, NDC, D], bf16)
            for t_sb, t_dr in ((wq_sb, wq), (wk_sb, wk), (wv_sb, wv), (dw_sb, dw)):
                nc.sync.dma_start(out=t_sb, in_=t_dr[:].rearrange("(o p) m -> p o m", p=128))
            bdm_sb = cpool.tile([128, 5, 128], bf16)
            nc.sync.dma_start(out=bdm_sb, in_=bdm[:].rearrange("t p m -> p t m"))
            ones_sb = cpool.tile([128, 128], bf16)
            nc.sync.dma_start(out=ones_sb, in_=onesbd[:])
            bias_sb = cpool.tile([128, D], f32)
            db_ap = db[:]
            nc.gpsimd.dma_start(
                out=bias_sb,
                in_=bass.AP(tensor=db_ap.tensor, offset=db_ap.offset,
                            ap=[[0, 128]] + list(db_ap.ap)))

            # Per-batch state; stages are emitted as closures so the two
            # batches can be interleaved in PE program order (engines execute
            # in order -- without interleaving, batch 1's QKV sits behind
            # batch 0's softmax gaps instead of filling them).
            st = [dict() for _ in range(BLOC)]

            def emit_x_load(b):
                s = st[b]
                s["xt"] = [bigp.tile([128, S], bf16, tag="xt", bufs=4,
                                     name=f"xt_{b}_{dc}")
                           for dc in range(NDC)]
                for dc in range(NDC):
                    nc.sync.dma_start(
                        out=s["xt"][dc],
                        in_=xT[b, 128 * dc:128 * (dc + 1), :])

            def emit_kv(b, j):
                s = st[b]
                if j == 0:
                    s["knat"] = bigp.tile([128, NJ, D], bf16, tag="knat",
                                          name=f"knat_{b}")
                    s["vnat"] = bigp.tile([128, NJ, D], bf16, tag="vnat",
                                          name=f"vnat_{b}")
                for w_sb, key in ((wk_sb, "knat"), (wv_sb, "vnat")):
                    ps_k = psB.tile([128, D], f32, tag="ps512")
                    for dc in range(NDC):
                        nc.tensor.matmul(
                            ps_k,
                            s["xt"][dc][:, 128 * j:128 * (j + 1)],
                            w_sb[:, dc, :],
                            start=(dc == 0), stop=(dc == NDC - 1))
                    nc.vector.tensor_copy(out=s[key][:, j, :], in_=ps_k)

            def emit_qt(b, pr, n):
                s = st[b]
                if pr == 0 and n == 0:
                    s["qt"] = bigp.tile([128, NPAIR, SCW * SCH + 4], bf16,
                                        tag="qT", bufs=2, name=f"qt_{b}")
                    nc.vector.memset(s["qt"][:, :, 0:2], 0.0)
                    nc.vector.memset(s["qt"][:, :, SCW * SCH + 2:], 0.0)
                ps_q = psB.tile([128, SCW], f32, tag="ps512")
                for dc in range(NDC):
                    nc.tensor.matmul(
                        ps_q,
                        wq_sb[:, dc, 128 * pr:128 * (pr + 1)],
                        s["xt"][dc][:, SCW * n:SCW * (n + 1)],
                        start=(dc == 0), stop=(dc == NDC - 1))
                nc.scalar.copy(
                    out=s["qt"][:, pr, 2 + SCW * n:2 + SCW * (n + 1)],
                    in_=ps_q)

            def emit_proj(b, pr):
                s = st[b]
                if pr == 0:
                    s["kp"] = bigp.tile([128, NPAIR, 128], bf16, tag="kpbd",
                                        bufs=2, name=f"kp_{b}")
                    s["vp"] = bigp.tile([128, NPAIR, 128], bf16, tag="vpbd",
                                        bufs=2, name=f"vp_{b}")
                    nc.vector.memset(s["kp"], 0.0)
                    nc.vector.memset(s["vp"], 0.0)
                ae_sb = ahp.tile([128, NJ, 128], bf16, tag="ae")
                af_sb = ahp.tile([128, NJ, 128], bf16, tag="af")
                nc.sync.dma_start(out=ae_sb, in_=ae[pr].rearrange("(j p) c -> p j c", p=128))
                nc.sync.dma_start(out=af_sb, in_=af[pr].rearrange("(j p) c -> p j c", p=128))
                for a_sb, key, dstk in ((ae_sb, "knat", "kp"), (af_sb, "vnat", "vp")):
                    # lhsT = [A_h0 | A_h1] columns, rhs = both heads' k/v
                    # columns; out diag blocks = the two k_proj's, off-diag
                    # blocks are cross-head garbage and are not copied.
                    ps_p = psS.tile([128, 128], f32, tag="pssmall")
                    for j in range(NJ):
                        nc.tensor.matmul(
                            ps_p,
                            a_sb[:, j, :],
                            st[b][key][:, j, 128 * pr:128 * (pr + 1)],
                            start=(j == 0), stop=(j == NJ - 1))
                    dst = st[b][dstk]
                    nc.vector.tensor_copy(
                        out=dst[0:64, pr, 0:64], in_=ps_p[0:64, 0:64])
                    nc.vector.tensor_copy(
                        out=dst[64:128, pr, 64:128], in_=ps_p[64:128, 64:128])

            def emit_kt(b, pr):
                s = st[b]
                if pr == 0:
                    s["concat"] = bigp.tile([128, NPAIR, S], bf16,
                                            tag="concatT", bufs=2,
                                            name=f"concat_{b}")
                    s["bdt"] = {}
                bdt = bdp.tile([128, 5, 128], bf16, tag="bdt",
                               name=f"bdt_{b}_{pr}")
                s["bdt"][pr] = bdt
                for t in range(5):
                    ps_b = psS.tile([128, 128], f32, tag="pssmall")
                    nc.tensor.matmul(ps_b, s["kp"][:, pr, :], bdm_sb[:, t, :],
                                     start=True, stop=True)
                    nc.vector.tensor_copy(out=bdt[:, t, :], in_=ps_b)

            def emit_att(b, pr, n):
                s = st[b]
                bdt = s["bdt"][pr]
                ps_sc = psB.tile([128, SCW], f32, tag="ps512")
                for ti in range(5):  # t = ti - 2
                    nc.tensor.matmul(
                        ps_sc,
                        bdt[:, ti, :],
                        s["qt"][:, pr, SCW * n + ti:SCW * n + ti + SCW],
                        start=(ti == 0), stop=(ti == 4))
                expt = smp.tile([128, SCW], bf16, tag="expt")
                nc.scalar.activation(
                    out=expt, in_=ps_sc,
                    func=mybir.ActivationFunctionType.Exp)
                ps_z = psB.tile([128, SCW], f32, tag="ps512")
                nc.tensor.matmul(ps_z, ones_sb, expt, start=True, stop=True)
                ps_at = psB.tile([128, SCW], f32, tag="ps512")
                nc.tensor.matmul(ps_at, vp_bd_of(s)[:, pr, :], expt,
                                 start=True, stop=True)
                # 1/Z: approx reciprocal (~18 bits, single DVE op). Exact
                # reciprocal is ~3.3us/tile; ACT ln/exp thrashes the table.
                rzb = smp.tile([128, SCW], f32, tag="rzb")
                nc.vector.reciprocal_approx_fast(out=rzb, in_=ps_z)
                nc.vector.tensor_mul(
                    out=s["concat"][:, pr, SCW * n:SCW * (n + 1)],
                    in0=ps_at, in1=rzb)

            def vp_bd_of(s):
                return s["vp"]

            def emit_dense(b, j):
                s = st[b]
                ps_d = psB.tile([128, D], f32, tag="ps512")
                for dc in range(NDC):
                    nc.tensor.matmul(
                        ps_d,
                        s["concat"][:, dc, 128 * j:128 * (j + 1)],
                        dw_sb[:, dc, :],
                        start=(dc == 0), stop=(dc == NDC - 1))
                obuf = obp.tile([128, D], f32, tag="obuf")
                nc.vector.tensor_add(out=obuf, in0=ps_d, in1=bias_sb)
                nc.sync.dma_start(out=out[b, 128 * j:128 * (j + 1), :],
                                  in_=obuf)

            # ---- emission schedule ----
            emit_x_load(0)
            for j in range(NJ):
                emit_kv(0, j)
            for pr in range(NPAIR):
                for n in range(SCH):
                    emit_qt(0, pr, n)
            for pr in range(NPAIR):
                emit_proj(0, pr)
            emit_x_load(1)
            # batch-0 attention interleaved with batch-1 kv+qT
            fillers = [(emit_kv, (1, j)) for j in range(NJ)] + \
                      [(emit_qt, (1, pr, n)) for pr in range(NPAIR)
                       for n in range(SCH)]
            fi = 0
            for pr in range(NPAIR):
                emit_kt(0, pr)
                for n in range(SCH):
                    emit_att(0, pr, n)
                    for _ in range(2):
                        if fi < len(fillers):
                            f, a = fillers[fi]; f(*a); fi += 1
            while fi < len(fillers):
                f, a = fillers[fi]; f(*a); fi += 1
            for pr in range(NPAIR):
                emit_proj(1, pr)
            # batch-1 attention interleaved with batch-0 dense
            dj = 0
            for pr in range(NPAIR):
                emit_kt(1, pr)
                for n in range(SCH):
                    emit_att(1, pr, n)
                    if dj < NJ:
                        emit_dense(0, dj); dj += 1
            while dj < NJ:
                emit_dense(0, dj); dj += 1
            for j in range(NJ):
                emit_dense(1, j)

    nc.finalize()
    return nc


def _prep_inputs(x, mask, wq, wk, wv, EW, FW, conv_w1, conv_w3, conv_w5, conv_b,
                 dense_w, dense_b, cluster_table):
    """Host-side restructuring -> per-core input maps."""
    bf = ml_dtypes.bfloat16
    x = np.ascontiguousarray(np.asarray(x, np.float32))
    mask = np.asarray(mask)
    counts = np.clip(mask.astype(np.int64).sum(1), 1, S)
    pos = np.asarray(cluster_table)[counts - 1]          # [B, P, C]
    if not (pos == pos[0]).all():
        raise NotImplementedError("per-batch cluster tables not supported")
    p0 = pos[0]                                          # [P, C]

    scale = 1.0 / np.sqrt(np.float32(DEPTH))
    s_idx = p0.ravel()
    c_idx = np.repeat(np.arange(P), C)

    def build_table(W, sc):
        A = np.zeros((H, S + 1, P), np.float32)
        np.add.at(A, (np.arange(H)[:, None], s_idx[None, :], c_idx[None, :]),
                  np.asarray(W, np.float32).reshape(H, P * C) * sc)
        return np.ascontiguousarray(A[:, :S, :])

    AE = build_table(EW, scale)
    AF = build_table(FW, 1.0)
    # pack adjacent heads side by side: [NPAIR, S, 128]
    AE = np.ascontiguousarray(
        AE.reshape(NPAIR, 2, S, P).transpose(0, 2, 1, 3).reshape(NPAIR, S, 128))
    AF = np.ascontiguousarray(
        AF.reshape(NPAIR, 2, S, P).transpose(0, 2, 1, 3).reshape(NPAIR, S, 128))

    # conv -> 5 tap matrices
    wp = np.arange(P)[:, None]
    jj = np.arange(P)[None, :]
    ii = wp - jj + 31
    valid = (ii >= 0) & (ii < P)
    ii = np.clip(ii, 0, P - 1)
    M = {t: np.zeros((P, P), np.float32) for t in range(-2, 3)}
    for cw, hk in ((conv_w1, 1), (conv_w3, 3), (conv_w5, 5)):
        cw = np.asarray(cw, np.float32)
        pad = (hk - 1) // 2
        for dy in range(hk):
            filt = cw[dy, :, 0, 0]
            M[dy - pad] += np.where(valid, filt[ii], 0.0) / 3.0
    BDM = np.zeros((5, 128, 128), np.float32)
    for ti in range(5):
        BDM[ti, :64, :64] = M[ti - 2]
        BDM[ti, 64:, 64:] = M[ti - 2]
    bbar = float(np.asarray(conv_b, np.float32).mean())
    if abs(bbar) > 1e-30:
        raise NotImplementedError("nonzero conv bias not folded")

    ones_bd = np.zeros((128, 128), np.float32)
    ones_bd[:64, :64] = 1.0
    ones_bd[64:, 64:] = 1.0

    # shard + transpose x
    xsh = x.reshape(NCORES, BLOC, S, D)
    in_maps = []
    shared = dict(
        wq=np.asarray(wq, np.float32).astype(bf),
        wk=np.asarray(wk, np.float32).astype(bf),
        wv=np.asarray(wv, np.float32).astype(bf),
        dw=np.asarray(dense_w, np.float32).astype(bf),
        db=np.asarray(dense_b, np.float32),
        ae=AE.astype(bf), af=AF.astype(bf), bdm=BDM.astype(bf),
        onesbd=ones_bd.astype(bf),
    )
    for c in range(NCORES):
        m = dict(shared)
        m["xT"] = np.ascontiguousarray(xsh[c].transpose(0, 2, 1)).astype(bf)
        in_maps.append(m)
    return in_maps


def _run(in_maps, trace=False, tmpdir=None):
    from concourse.bass_utils import run_bass_kernel_spmd
    if "nc" not in _CACHE:
        _CACHE["nc"] = _build_nc()
    kw = {}
    if trace:
        _install_ntff_hook()
        kw = dict(trace=True, tmpdir=tmpdir)
    return run_bass_kernel_spmd(_CACHE["nc"], in_maps,
                                core_ids=list(range(NCORES)), **kw)


def _install_ntff_hook():
    import types, importlib.util as ilu
    if "antenv.axon_hooks" in sys.modules:
        return
    spec = ilu.spec_from_file_location(
        "trn_boot_mod", "/root/.axon_site/trn_agent_boot/trn_boot.py")
    tb = ilu.module_from_spec(spec)
    spec.loader.exec_module(tb)
    hook = tb._ntff_profile_via_ctypes("/opt/axon/libaxon_pjrt.so")
    mod = types.ModuleType("antenv.axon_hooks")
    mod.get_axon_ntff_profile_hook = lambda: hook
    import antenv  # noqa: F401
    sys.modules["antenv.axon_hooks"] = mod


def kernel(**inputs) -> np.ndarray:
    in_maps = _prep_inputs(**inputs)
    r = _run(in_maps)
    return np.concatenate([r.results[c]["out"] for c in range(NCORES)], axis=0)
